# revision 9
# baseline (speedup 1.0000x reference)
"""Trainium2 Bass kernel for nn_DecoderBlock (pre-LN causal attention + MLP).

Sharding: DP=2 over batch x TP=4 (heads / n_inner) across 8 NeuronCores.
Core c = 4*b + r handles batch b, head group r (4 heads of DH=128), and MLP
inner slice r. Activations are kept feature-on-partition ("transposed") on
device; LayerNorm column stats come from ones-matmuls over the partition dim;
matmuls run in float32r (fp32 inputs rounded, fp32 PSUM accumulation).
Collectives (per 4-core group): AllGather of the head-concat attention
output, ReduceScatter after dense2. Host only slices/transposes inputs and
concatenates per-core output shards.
"""

import contextlib
import math

import numpy as np

import concourse.bass as bass
import concourse.mybir as mybir
import concourse.tile as tile
from concourse import bacc
from concourse.bass_utils import run_bass_kernel_spmd
from concourse.masks import make_causal_mask, make_identity

FP32 = mybir.dt.float32
FP32R = mybir.dt.float32r
AF = mybir.ActivationFunctionType
OP = mybir.AluOpType
AX = mybir.AxisListType
ts = bass.ts
P = 128

B, T, D, H, F = 2, 2048, 2048, 16, 8192
DH = 128
EPS = 1e-6
NCORES = 8
TP = 4
GROUPS = [[0, 1, 2, 3], [4, 5, 6, 7]]
GELU_FUNC = AF.Gelu_apprx_tanh  # sim overrides: not implemented in bass_interp


class Cfg:
    def __init__(self, T, D, F, HL):
        self.T = T              # tokens per core
        self.D = D              # embed dim
        self.FL = F // TP       # local mlp inner dim
        self.HL = HL            # local heads
        self.DC = D // P        # D 128-chunks
        self.FC = self.FL // P  # FL 128-chunks
        self.TT = T // P        # token 128-chunks
        self.TN = T // 512      # token 512-chunks
        self.QKM = 2 * HL       # qT+kT row chunks
        self.DL = D // TP       # output rows per core
        self.VW = HL * DH       # v columns


FULL = Cfg(T, D, F, H // TP)


def _layernorm_T(nc, tc, chunks, ones, g_sb, b_sb, cfg):
    """LayerNorm over the partition (D) axis of transposed chunks, in place.

    chunks: DC SBUF tiles [128, T] fp32r covering D; on return they hold
    (x - mu) * rsqrt(var + eps) * gamma + beta in fp32r.
    """
    Tl, DC, TN = cfg.T, cfg.DC, cfg.TN
    with (
        tc.tile_pool(name="ln_sb", bufs=1) as stats,
        tc.tile_pool(name="ln_sq", bufs=2) as sqp,
        tc.tile_pool(name="ln_ps", bufs=1, space="PSUM") as stat_ps,
    ):
        s_sum = stats.tile([1, Tl], FP32, tag="sum")
        s_sq = stats.tile([1, Tl], FP32, tag="sq")
        for tn in range(TN):
            ps_a = stat_ps.tile([1, 512], FP32, tag="psa")
            ps_b = stat_ps.tile([1, 512], FP32, tag="psb")
            for dc in range(DC):
                sq = sqp.tile([P, 512], FP32R, tag="sqc")
                nc.scalar.activation(sq[:], chunks[dc][:, ts(tn, 512)], AF.Square)
                nc.tensor.matmul(
                    ps_a[:], ones[:], chunks[dc][:, ts(tn, 512)],
                    start=(dc == 0), stop=(dc == DC - 1),
                )
                nc.tensor.matmul(
                    ps_b[:], ones[:], sq[:], start=(dc == 0), stop=(dc == DC - 1)
                )
            nc.vector.tensor_copy(s_sum[:, ts(tn, 512)], ps_a[:])
            nc.vector.tensor_copy(s_sq[:, ts(tn, 512)], ps_b[:])
        inv = stats.tile([1, Tl], FP32, tag="inv")
        mu_pl = stats.tile([P, Tl], FP32, tag="mupl")
        inv_pl = stats.tile([P, Tl], FP32, tag="invpl")
        mu = s_sum  # in-place: sum -> mean
        nc.vector.tensor_scalar_mul(mu[:], s_sum[:], 1.0 / cfg.D)
        nc.vector.tensor_scalar_mul(s_sq[:], s_sq[:], 1.0 / cfg.D)
        mu2 = mu_pl[:1, :]  # scratch row inside the plane buffer
        nc.vector.tensor_tensor(mu2, mu[:], mu[:], op=OP.mult)
        nc.vector.tensor_tensor(s_sq[:], s_sq[:], mu2, op=OP.subtract)
        nc.vector.tensor_scalar_add(s_sq[:], s_sq[:], EPS)
        nc.scalar.activation(s_sq[:], s_sq[:], AF.Sqrt)
        nc.vector.reciprocal(inv[:], s_sq[:])
        nc.gpsimd.partition_broadcast(inv_pl[:], inv[:])
        nc.gpsimd.partition_broadcast(mu_pl[:], mu[:])
        for dc in range(DC):
            c = chunks[dc]
            nc.vector.tensor_tensor(c[:], c[:], mu_pl[:], op=OP.subtract)
            nc.vector.tensor_tensor(c[:], c[:], inv_pl[:], op=OP.mult)
            nc.vector.tensor_scalar(
                c[:], c[:], g_sb[:, dc : dc + 1], b_sb[:, dc : dc + 1],
                op0=OP.mult, op1=OP.add,
            )


def build(cfg=FULL):
    Tl, Dd, FL, HL = cfg.T, cfg.D, cfg.FL, cfg.HL
    DC, FC, TT, TN, QKM = cfg.DC, cfg.FC, cfg.TT, cfg.TN, cfg.QKM
    DL, VW = cfg.DL, cfg.VW
    scale = 1.0 / math.sqrt(DH)
    assert Tl == Dd, "w2 cache reuses [P, Tl] tiles"

    nc = bacc.Bacc(None, target_bir_lowering=False, num_devices=NCORES)

    # ---- external I/O (per core) ----
    xt = nc.declare_dram_parameter("xt", [Dd, Tl], FP32, isOutput=False)
    wqkv = nc.declare_dram_parameter("wqkv", [Dd, QKM * P + VW], FP32,
                                     isOutput=False)
    bqk = nc.declare_dram_parameter("bqk", [P, QKM], FP32, isOutput=False)
    bv = nc.declare_dram_parameter("bv", [1, VW], FP32, isOutput=False)
    ln1g = nc.declare_dram_parameter("ln1g", [P, DC], FP32, isOutput=False)
    ln1b = nc.declare_dram_parameter("ln1b", [P, DC], FP32, isOutput=False)
    ln2g = nc.declare_dram_parameter("ln2g", [P, DC], FP32, isOutput=False)
    ln2b = nc.declare_dram_parameter("ln2b", [P, DC], FP32, isOutput=False)
    w1 = nc.declare_dram_parameter("w1", [Dd, FL], FP32, isOutput=False)
    b1 = nc.declare_dram_parameter("b1", [P, FC], FP32, isOutput=False)
    w2 = nc.declare_dram_parameter("w2", [FL, Dd], FP32, isOutput=False)
    b2 = nc.declare_dram_parameter("b2", [P, DL // P], FP32, isOutput=False)
    xres = nc.declare_dram_parameter("xres", [DL, Tl], FP32, isOutput=False)
    out = nc.declare_dram_parameter("out", [DL, Tl], FP32, isOutput=True)

    with tile.TileContext(nc) as tc:
        ctx = contextlib.ExitStack()
        with ctx:
            dram = ctx.enter_context(
                tc.tile_pool(name="dramstage", bufs=1, space="DRAM")
            )
            qkT_d = dram.tile([QKM * P, Tl], FP32R, tag="qkT")
            v_d = dram.tile([TT * P, VW], FP32R, tag="v")
            oT_d = dram.tile([HL * DH, Tl], FP32, tag="oT")
            ag_d = dram.tile([TP * HL * DH, Tl], FP32, tag="ag")
            h_d = dram.tile([FL, Tl], FP32R, tag="h")
            rs_in_d = dram.tile([Dd, Tl], FP32, tag="rsin")
            rs_out_d = dram.tile([DL, Tl], FP32, tag="rsout")

            const = ctx.enter_context(tc.tile_pool(name="const", bufs=1))
            ones32 = const.tile([P, 1], FP32, tag="ones32")
            nc.vector.memset(ones32[:], 1.0)
            ones = const.tile([P, 1], FP32R, tag="ones")
            nc.vector.tensor_copy(ones[:], ones32[:])
            cmask = const.tile([P, P], FP32, tag="cmask")
            make_causal_mask(nc, cmask[:], mask_val=-1e10)
            ident32 = const.tile([P, P], FP32, tag="ident32")
            make_identity(nc, ident32[:])
            ident = const.tile([P, P], FP32R, tag="ident")
            nc.vector.tensor_copy(ident[:], ident32[:])
            ln_g1 = const.tile([P, DC], FP32, tag="g1")
            ln_b1 = const.tile([P, DC], FP32, tag="lb1")
            ln_g2 = const.tile([P, DC], FP32, tag="g2")
            ln_b2 = const.tile([P, DC], FP32, tag="lb2")
            bqk_sb = const.tile([P, QKM], FP32, tag="bqk")
            bv_row = const.tile([1, VW], FP32, tag="bvr")
            bv_pl = const.tile([P, VW], FP32, tag="bvp")
            b1_sb = const.tile([P, FC], FP32, tag="b1s")
            b2_sb = const.tile([P, DL // P], FP32, tag="b2s")
            nc.sync.dma_start(ln_g1[:], ln1g[:])
            nc.sync.dma_start(ln_b1[:], ln1b[:])
            nc.sync.dma_start(ln_g2[:], ln2g[:])
            nc.sync.dma_start(ln_b2[:], ln2b[:])
            nc.sync.dma_start(bqk_sb[:], bqk[:])
            nc.sync.dma_start(bv_row[:], bv[:])
            nc.sync.dma_start(b1_sb[:], b1[:])
            nc.sync.dma_start(b2_sb[:], b2[:])
            nc.gpsimd.partition_broadcast(bv_pl[:], bv_row[:])

            # ======== Phase A: LN1 (x^T -> r^T in place, SBUF) ========
            big = tc.alloc_tile_pool(name="big", bufs=1)
            rT = []
            for dc in range(DC):
                t = big.tile([P, Tl], FP32R, tag=f"big{dc}")
                nc.gpsimd.dma_start(t[:], xt[ts(dc, P), :])
                rT.append(t)
            _layernorm_T(nc, tc, rT, ones, ln_g1, ln_b1, cfg)

            # ======== Phase B: qkv projections ========
            with (
                tc.tile_pool(name="ev", bufs=3) as ev,
                tc.tile_pool(name="wvp", bufs=1) as wvp,
                tc.tile_pool(name="wpool", bufs=2) as wpool,
                tc.tile_pool(name="bps", bufs=2, space="PSUM") as bps,
            ):
                for mc in range(QKM):
                    wt = wpool.tile([P, DC, P], FP32R, tag="wqk")
                    nc.gpsimd.dma_start(
                        wt[:],
                        wqkv[:, ts(mc, P)].rearrange("(c k) m -> k c m", k=P),
                    )
                    for tn in range(TN):
                        acc = bps.tile([P, 512], FP32, tag="acc")
                        for dc in range(DC):
                            nc.tensor.matmul(
                                acc[:], wt[:, dc, :], rT[dc][:, ts(tn, 512)],
                                start=(dc == 0), stop=(dc == DC - 1),
                            )
                        st = ev.tile([P, 512], FP32R, tag="qk_ev")
                        nc.vector.tensor_scalar_add(
                            st[:], acc[:], bqk_sb[:, mc : mc + 1]
                        )
                        nc.sync.dma_start(qkT_d[ts(mc, P), ts(tn, 512)], st[:])

                # v natural [T, VW]: lhsT = rT token chunk, rhs = w_v K-chunks
                nvw = (VW + 511) // 512
                for vn in range(nvw):
                    w = min(512, VW - vn * 512)
                    wv = wvp.tile([P, DC, 512], FP32R, tag="wv")
                    nc.gpsimd.dma_start(
                        wv[:, :, :w],
                        wqkv[:, QKM * P + vn * 512 : QKM * P + vn * 512 + w]
                        .rearrange("(c k) m -> k c m", k=P),
                    )
                    for tck in range(TT):
                        acc = bps.tile([P, 512], FP32, tag="acc")
                        for dc in range(DC):
                            nc.tensor.matmul(
                                acc[:, :w], rT[dc][:, ts(tck, P)],
                                wv[:, dc, :w],
                                start=(dc == 0), stop=(dc == DC - 1),
                            )
                        ve = ev.tile([P, 512], FP32R, tag="v_ev")
                        nc.vector.tensor_tensor(
                            ve[:, :w], acc[:, :w],
                            bv_pl[:, vn * 512 : vn * 512 + w], op=OP.add,
                        )
                        nc.sync.dma_start(
                            v_d[ts(tck, P), vn * 512 : vn * 512 + w], ve[:, :w]
                        )
            big.release()  # r^T dead; free 16MB before attention

            # ======== Phase C: causal attention per head ========
            with (
                tc.tile_pool(name="apool", bufs=2) as apool,
                tc.tile_pool(name="vhp", bufs=2) as vhp,
                tc.tile_pool(name="ppool", bufs=2) as ppool,
                tc.tile_pool(name="ptasm", bufs=1) as ptasm,
                tc.tile_pool(name="small", bufs=6) as small,
                tc.tile_pool(name="opool", bufs=1) as opool,
                tc.tile_pool(name="s_ps", bufs=4, space="PSUM") as s_ps,
                tc.tile_pool(name="t_ps", bufs=2, space="PSUM") as t_ps,
                tc.tile_pool(name="o_ps", bufs=2, space="PSUM") as o_ps,
            ):
                oT_sb = [opool.tile([P, Tl], FP32, tag=f"o{h}", name=f"oT{h}") for h in range(HL)]
                for h in range(HL):
                    qT = apool.tile([P, Tl], FP32R, tag="qT")
                    kT = apool.tile([P, Tl], FP32R, tag="kT")
                    nc.sync.dma_start(qT[:], qkT_d[ts(h, P), :])
                    nc.sync.dma_start(kT[:], qkT_d[ts(HL + h, P), :])
                    vh = []
                    for k in range(TT):
                        vt = vhp.tile([P, P], FP32R, tag=f"vh{k}")
                        nc.sync.dma_start(vt[:], v_d[ts(k, P), ts(h, P)])
                        vh.append(vt)
                    for jj in range(TN):
                        pts = [
                            ptasm.tile([P, 512], FP32R, tag=f"pt{k}",
                                       name=f"pt{k}")
                            for k in range(4 * jj + 4)
                        ]
                        for i in range(4 * jj, 4 * jj + 4):
                            nk = i + 1
                            ng = (nk + 3) // 4
                            prow = ppool.tile([P, Tl], FP32R, tag="prow")
                            sg = []
                            for g in range(ng):
                                w = min(512, nk * P - g * 512)
                                st = s_ps.tile([P, 512], FP32, tag="s")
                                nc.tensor.matmul(
                                    st[:, :w], qT[:, ts(i, P)],
                                    kT[:, g * 512 : g * 512 + w],
                                    start=True, stop=True,
                                )
                                sg.append((st, w))
                            dst, dw = sg[-1]
                            nc.vector.tensor_tensor(
                                dst[:, dw - P : dw], dst[:, dw - P : dw],
                                cmask[:], op=OP.add,
                            )
                            mx = small.tile([P, 1], FP32, tag="mx")
                            for g, (st, w) in enumerate(sg):
                                if g == 0:
                                    nc.vector.tensor_reduce(
                                        mx[:], st[:, :w], axis=AX.X, op=OP.max
                                    )
                                else:
                                    m2 = small.tile([P, 1], FP32, tag="mx2")
                                    nc.vector.tensor_reduce(
                                        m2[:], st[:, :w], axis=AX.X, op=OP.max
                                    )
                                    nc.vector.tensor_tensor(
                                        mx[:], mx[:], m2[:], op=OP.max
                                    )
                            nbias = small.tile([P, 1], FP32, tag="nb")
                            nc.vector.tensor_scalar_mul(nbias[:], mx[:], -scale)
                            tot = small.tile([P, 1], FP32, tag="tot")
                            for g, (st, w) in enumerate(sg):
                                acc_o = small.tile([P, 1], FP32, tag="acc_o")
                                nc.scalar.activation(
                                    prow[:, g * 512 : g * 512 + w], st[:, :w],
                                    AF.Exp, bias=nbias[:], scale=scale,
                                    accum_out=acc_o[:],
                                )
                                if g == 0:
                                    nc.vector.tensor_copy(tot[:], acc_o[:])
                                else:
                                    nc.vector.tensor_tensor(
                                        tot[:], tot[:], acc_o[:], op=OP.add
                                    )
                            rcp = small.tile([P, 1], FP32, tag="rcp")
                            nc.vector.reciprocal(rcp[:], tot[:])
                            nc.vector.tensor_scalar_mul(
                                prow[:, : nk * P], prow[:, : nk * P], rcp[:]
                            )
                            col = (i - 4 * jj) * P
                            for k in range(nk):
                                tp = t_ps.tile([P, P], FP32R, tag="tp")
                                nc.tensor.transpose(
                                    tp[:], prow[:, ts(k, P)], ident[:]
                                )
                                nc.vector.tensor_copy(
                                    pts[k][:, col : col + P], tp[:]
                                )
                        op_t = o_ps.tile([P, 512], FP32, tag="o")
                        last = 4 * jj + 3
                        for k in range(4 * jj + 4):
                            c0 = max(0, (k - 4 * jj) * P)
                            nc.tensor.matmul(
                                op_t[:, c0:], vh[k][:], pts[k][:, c0:],
                                start=(k == 0), stop=(k == last),
                            )
                        nc.vector.tensor_copy(oT_sb[h][:, ts(jj, 512)], op_t[:])
                for h in range(HL):
                    nc.sync.dma_start(oT_d[ts(h, P), :], oT_sb[h][:])

            # AllGather head-concat attention output across the TP group
            nc.gpsimd.collective_compute(
                "AllGather", OP.bypass, replica_groups=GROUPS,
                ins=[oT_d.opt()], outs=[ag_d.opt()],
            )

            # ======== Phase D: x2 = x + attn, LN2 -> r2^T (SBUF) ========
            big2 = tc.alloc_tile_pool(name="big2", bufs=1)
            r2 = []
            with tc.tile_pool(name="ldp", bufs=2) as ldp:
                for dc in range(DC):
                    t = big2.tile([P, Tl], FP32R, tag=f"bg{dc}")
                    for tn in range(TN):
                        xa = ldp.tile([P, 512], FP32, tag="xa")
                        xb = ldp.tile([P, 512], FP32, tag="xb")
                        nc.sync.dma_start(xa[:], xt[ts(dc, P), ts(tn, 512)])
                        nc.sync.dma_start(xb[:], ag_d[ts(dc, P), ts(tn, 512)])
                        nc.vector.tensor_tensor(
                            t[:, ts(tn, 512)], xa[:], xb[:], op=OP.add
                        )
                    r2.append(t)
            _layernorm_T(nc, tc, r2, ones, ln_g2, ln_b2, cfg)

            # ======== Phase E: h = gelu(r2 @ w1 + b1) -> h_d ========
            with (
                tc.tile_pool(name="eev", bufs=3) as eev,
                tc.tile_pool(name="wpoole", bufs=2) as wpool,
                tc.tile_pool(name="eps", bufs=2, space="PSUM") as eps_p,
            ):
                for fc in range(FC):
                    wt = wpool.tile([P, DC, P], FP32R, tag="wqk")
                    nc.gpsimd.dma_start(
                        wt[:],
                        w1[:, ts(fc, P)].rearrange("(c k) m -> k c m", k=P),
                    )
                    for tn in range(TN):
                        acc = eps_p.tile([P, 512], FP32, tag="acc")
                        for dc in range(DC):
                            nc.tensor.matmul(
                                acc[:], wt[:, dc, :], r2[dc][:, ts(tn, 512)],
                                start=(dc == 0), stop=(dc == DC - 1),
                            )
                        hv = eev.tile([P, 512], FP32R, tag="hev")
                        nc.scalar.activation(
                            hv[:], acc[:], GELU_FUNC,
                            bias=b1_sb[:, fc : fc + 1],
                        )
                        nc.sync.dma_start(h_d[ts(fc, P), ts(tn, 512)], hv[:])
            big2.release()

            # ======== Phase F: partial y^T = h @ w2 -> rs_in ========
            with (
                tc.tile_pool(name="w2c", bufs=1) as w2c,
                tc.tile_pool(name="hsp", bufs=1) as hsp,
                tc.tile_pool(name="fev", bufs=3) as fev,
                tc.tile_pool(name="fps", bufs=2, space="PSUM") as fps,
            ):
                w2t = []
                for fc in range(FC):
                    t = w2c.tile([P, Dd], FP32R, tag=f"w2{fc}")
                    nc.gpsimd.dma_start(t[:], w2[ts(fc, P), :])
                    w2t.append(t)
                for tn in range(TN):
                    hs = [
                        hsp.tile([P, 512], FP32R, tag=f"hs{f_}", name=f"hs{f_}")
                        for f_ in range(FC)
                    ]
                    for fc in range(FC):
                        nc.sync.dma_start(hs[fc][:], h_d[ts(fc, P), ts(tn, 512)])
                    for mc in range(DC):
                        acc = fps.tile([P, 512], FP32, tag="acc")
                        for fc in range(FC):
                            nc.tensor.matmul(
                                acc[:], w2t[fc][:, ts(mc, P)], hs[fc][:],
                                start=(fc == 0), stop=(fc == FC - 1),
                            )
                        yv = fev.tile([P, 512], FP32, tag="yev")
                        nc.vector.tensor_copy(yv[:], acc[:])
                        nc.sync.dma_start(rs_in_d[ts(mc, P), ts(tn, 512)], yv[:])

            nc.gpsimd.collective_compute(
                "ReduceScatter", OP.add, replica_groups=GROUPS,
                ins=[rs_in_d.opt()], outs=[rs_out_d.opt()],
            )

            # ======== Final: out = rs + x_res + own attn rows + b2 ========
            with tc.tile_pool(name="fin", bufs=2) as fin:
                for q in range(DL // P):
                    a = fin.tile([P, Tl], FP32, tag="fa")
                    bt = fin.tile([P, Tl], FP32, tag="fb")
                    c = fin.tile([P, Tl], FP32, tag="fc")
                    nc.sync.dma_start(a[:], rs_out_d[ts(q, P), :])
                    nc.sync.dma_start(bt[:], xres[ts(q, P), :])
                    nc.sync.dma_start(c[:], oT_d[ts(q, P), :])
                    nc.vector.tensor_tensor(a[:], a[:], bt[:], op=OP.add)
                    nc.vector.tensor_tensor(a[:], a[:], c[:], op=OP.add)
                    nc.vector.tensor_scalar_add(a[:], a[:], b2_sb[:, q : q + 1])
                    nc.sync.dma_start(out[ts(q, P), :], a[:])

    nc.compile()
    return nc


_NC_CACHE = {}


def _get_nc(cfg=FULL):
    key = (cfg.T, cfg.D, cfg.FL, cfg.HL)
    if key not in _NC_CACHE:
        _NC_CACHE[key] = build(cfg)
    return _NC_CACHE[key]


def _prep_inputs(x, ln1_scale, ln1_bias, w_qkv, b_qkv, ln2_scale, ln2_bias,
                 w1, b1, w2, b2, cfg=FULL):
    """Host-side sharding: per-core input dicts (core = 4*b + r)."""
    f32 = np.float32
    Dd, FL, HL, DC, FC, QKM, DL, VW = (cfg.D, cfg.FL, cfg.HL, cfg.DC, cfg.FC,
                                       cfg.QKM, cfg.DL, cfg.VW)

    def colmaj(v, nch):
        return np.ascontiguousarray(np.asarray(v, f32).reshape(nch, P).T)

    ln1g = colmaj(ln1_scale, DC)
    ln1b_ = colmaj(ln1_bias, DC)
    ln2g = colmaj(ln2_scale, DC)
    ln2b_ = colmaj(ln2_bias, DC)
    x = np.asarray(x, f32)
    w_qkv = np.asarray(w_qkv, f32)
    b_qkv = np.asarray(b_qkv, f32)
    w1 = np.asarray(w1, f32)
    w2 = np.asarray(w2, f32)
    b1 = np.asarray(b1, f32)
    b2 = np.asarray(b2, f32)

    in_maps = []
    for core in range(NCORES):
        b_, r = divmod(core, TP)
        qs, ks, vs = (r * VW, Dd + r * VW, 2 * Dd + r * VW)
        wqkv_s = np.ascontiguousarray(
            np.concatenate(
                [w_qkv[:, qs : qs + VW], w_qkv[:, ks : ks + VW],
                 w_qkv[:, vs : vs + VW]], axis=1,
            )
        )
        bqk_s = colmaj(
            np.concatenate([b_qkv[qs : qs + VW], b_qkv[ks : ks + VW]]), QKM
        )
        bv_s = np.ascontiguousarray(b_qkv[vs : vs + VW].reshape(1, VW))
        in_maps.append({
            "xt": np.ascontiguousarray(x[b_].T),
            "wqkv": wqkv_s,
            "bqk": bqk_s,
            "bv": bv_s,
            "ln1g": ln1g, "ln1b": ln1b_, "ln2g": ln2g, "ln2b": ln2b_,
            "w1": np.ascontiguousarray(w1[:, r * FL : (r + 1) * FL]),
            "b1": colmaj(b1[r * FL : (r + 1) * FL], FC),
            "w2": np.ascontiguousarray(w2[r * FL : (r + 1) * FL, :]),
            "b2": colmaj(b2[r * DL : (r + 1) * DL], DL // P),
            "xres": np.ascontiguousarray(x[b_][:, r * DL : (r + 1) * DL].T),
        })
    return in_maps


def kernel(**inputs):
    cfg = FULL
    nc = _get_nc(cfg)
    in_maps = _prep_inputs(**inputs, cfg=cfg)
    res = run_bass_kernel_spmd(nc, in_maps, core_ids=list(range(NCORES)))
    y = np.empty((B, cfg.T, cfg.D), np.float32)
    for b_ in range(B):
        yt = np.concatenate(
            [res.results[4 * b_ + r]["out"] for r in range(TP)], axis=0
        )
        y[b_] = yt.T
    return y


# revision 11
# speedup vs baseline: 1.0827x; 1.0827x over previous
"""Trainium2 Bass kernel for nn_DecoderBlock (pre-LN causal attention + MLP).

Sharding: DP=2 over batch x TP=4 (heads / n_inner) across 8 NeuronCores.
Core c = 4*b + r handles batch b, head group r (4 heads of DH=128), and MLP
inner slice r. Activations are kept feature-on-partition ("transposed") on
device; LayerNorm column stats come from ones-matmuls over the partition dim;
matmuls run in float32r (fp32 inputs rounded, fp32 PSUM accumulation).
Collectives (per 4-core group): AllGather of the head-concat attention
output, ReduceScatter after dense2. Host only slices/transposes inputs and
concatenates per-core output shards.
"""

import contextlib
import math

import numpy as np

import concourse.bass as bass
import concourse.mybir as mybir
import concourse.tile as tile
from concourse import bacc
from concourse.bass_utils import run_bass_kernel_spmd
from concourse.masks import make_causal_mask, make_identity

FP32 = mybir.dt.float32
FP32R = mybir.dt.float32r
AF = mybir.ActivationFunctionType
OP = mybir.AluOpType
AX = mybir.AxisListType
ts = bass.ts
P = 128

B, T, D, H, F = 2, 2048, 2048, 16, 8192
DH = 128
EPS = 1e-6
NCORES = 8
TP = 4
GROUPS = [[0, 1, 2, 3], [4, 5, 6, 7]]
GELU_FUNC = AF.Gelu_apprx_tanh  # sim overrides: not implemented in bass_interp


class Cfg:
    def __init__(self, T, D, F, HL):
        self.T = T              # tokens per core
        self.D = D              # embed dim
        self.FL = F // TP       # local mlp inner dim
        self.HL = HL            # local heads
        self.DC = D // P        # D 128-chunks
        self.FC = self.FL // P  # FL 128-chunks
        self.TT = T // P        # token 128-chunks
        self.TN = T // 512      # token 512-chunks
        self.QKM = 2 * HL       # qT+kT row chunks
        self.DL = D // TP       # output rows per core
        self.VW = HL * DH       # v columns


FULL = Cfg(T, D, F, H // TP)


def _layernorm_T(nc, tc, chunks, ones, g_sb, b_sb, cfg):
    """LayerNorm over the partition (D) axis of transposed chunks, in place.

    chunks: DC SBUF tiles [128, T] fp32r covering D; on return they hold
    (x - mu) * rsqrt(var + eps) * gamma + beta in fp32r.
    """
    Tl, DC, TN = cfg.T, cfg.DC, cfg.TN
    with (
        tc.tile_pool(name="ln_sb", bufs=1) as stats,
        tc.tile_pool(name="ln_sq", bufs=2) as sqp,
        tc.tile_pool(name="ln_ps", bufs=1, space="PSUM") as stat_ps,
    ):
        s_sum = stats.tile([1, Tl], FP32, tag="sum")
        s_sq = stats.tile([1, Tl], FP32, tag="sq")
        for tn in range(TN):
            ps_a = stat_ps.tile([1, 512], FP32, tag="psa")
            ps_b = stat_ps.tile([1, 512], FP32, tag="psb")
            for dc in range(DC):
                sq = sqp.tile([P, 512], FP32R, tag="sqc")
                nc.scalar.activation(sq[:], chunks[dc][:, ts(tn, 512)], AF.Square)
                nc.tensor.matmul(
                    ps_a[:], ones[:], chunks[dc][:, ts(tn, 512)],
                    start=(dc == 0), stop=(dc == DC - 1),
                )
                nc.tensor.matmul(
                    ps_b[:], ones[:], sq[:], start=(dc == 0), stop=(dc == DC - 1)
                )
            nc.vector.tensor_copy(s_sum[:, ts(tn, 512)], ps_a[:])
            nc.vector.tensor_copy(s_sq[:, ts(tn, 512)], ps_b[:])
        inv = stats.tile([1, Tl], FP32, tag="inv")
        mu_pl = stats.tile([P, Tl], FP32, tag="mupl")
        inv_pl = stats.tile([P, Tl], FP32, tag="invpl")
        mu = s_sum  # in-place: sum -> mean
        nc.vector.tensor_scalar_mul(mu[:], s_sum[:], 1.0 / cfg.D)
        nc.vector.tensor_scalar_mul(s_sq[:], s_sq[:], 1.0 / cfg.D)
        mu2 = mu_pl[:1, :]  # scratch row inside the plane buffer
        nc.vector.tensor_tensor(mu2, mu[:], mu[:], op=OP.mult)
        nc.vector.tensor_tensor(s_sq[:], s_sq[:], mu2, op=OP.subtract)
        nc.vector.tensor_scalar_add(s_sq[:], s_sq[:], EPS)
        nc.scalar.activation(s_sq[:], s_sq[:], AF.Sqrt)
        nc.vector.reciprocal(inv[:], s_sq[:])
        nc.gpsimd.partition_broadcast(inv_pl[:], inv[:])
        nc.gpsimd.partition_broadcast(mu_pl[:], mu[:])
        for dc in range(DC):
            c = chunks[dc]
            nc.vector.tensor_tensor(c[:], c[:], mu_pl[:], op=OP.subtract)
            nc.vector.tensor_tensor(c[:], c[:], inv_pl[:], op=OP.mult)
            nc.vector.tensor_scalar(
                c[:], c[:], g_sb[:, dc : dc + 1], b_sb[:, dc : dc + 1],
                op0=OP.mult, op1=OP.add,
            )


def build(cfg=FULL):
    Tl, Dd, FL, HL = cfg.T, cfg.D, cfg.FL, cfg.HL
    DC, FC, TT, TN, QKM = cfg.DC, cfg.FC, cfg.TT, cfg.TN, cfg.QKM
    DL, VW = cfg.DL, cfg.VW
    scale = 1.0 / math.sqrt(DH)
    assert Tl == Dd, "w2 cache reuses [P, Tl] tiles"

    nc = bacc.Bacc(None, target_bir_lowering=False, num_devices=NCORES)

    # ---- external I/O (per core) ----
    xt = nc.declare_dram_parameter("xt", [Dd, Tl], FP32, isOutput=False)
    wqkv = nc.declare_dram_parameter("wqkv", [Dd, QKM * P + VW], FP32,
                                     isOutput=False)
    bqk = nc.declare_dram_parameter("bqk", [P, QKM], FP32, isOutput=False)
    bv = nc.declare_dram_parameter("bv", [1, VW], FP32, isOutput=False)
    ln1g = nc.declare_dram_parameter("ln1g", [P, DC], FP32, isOutput=False)
    ln1b = nc.declare_dram_parameter("ln1b", [P, DC], FP32, isOutput=False)
    ln2g = nc.declare_dram_parameter("ln2g", [P, DC], FP32, isOutput=False)
    ln2b = nc.declare_dram_parameter("ln2b", [P, DC], FP32, isOutput=False)
    w1 = nc.declare_dram_parameter("w1", [Dd, FL], FP32, isOutput=False)
    b1 = nc.declare_dram_parameter("b1", [P, FC], FP32, isOutput=False)
    w2 = nc.declare_dram_parameter("w2", [FL, Dd], FP32, isOutput=False)
    b2 = nc.declare_dram_parameter("b2", [P, DL // P], FP32, isOutput=False)
    xres = nc.declare_dram_parameter("xres", [DL, Tl], FP32, isOutput=False)
    out = nc.declare_dram_parameter("out", [DL, Tl], FP32, isOutput=True)

    with tile.TileContext(nc) as tc:
        ctx = contextlib.ExitStack()
        with ctx:
            dram = ctx.enter_context(
                tc.tile_pool(name="dramstage", bufs=1, space="DRAM")
            )
            qkT_d = dram.tile([QKM * P, Tl], FP32R, tag="qkT")
            v_d = dram.tile([TT * P, VW], FP32R, tag="v")
            oT_hd = [
                dram.tile([DH, Tl], FP32, tag=f"oT{h}", name=f"oT_hd{h}")
                for h in range(HL)
            ]
            ag_hd = [
                dram.tile([TP * DH, Tl], FP32, tag=f"ag{h}", name=f"ag_hd{h}")
                for h in range(HL)
            ]
            h_d = dram.tile([FL, Tl], FP32R, tag="h")
            rs_in_td = [
                dram.tile([Dd, 512], FP32, tag=f"rsin{t_}", name=f"rs_in{t_}")
                for t_ in range(TN)
            ]
            rs_out_td = [
                dram.tile([DL, 512], FP32, tag=f"rsout{t_}", name=f"rs_out{t_}")
                for t_ in range(TN)
            ]

            const = ctx.enter_context(tc.tile_pool(name="const", bufs=1))
            ones32 = const.tile([P, 1], FP32, tag="ones32")
            nc.vector.memset(ones32[:], 1.0)
            ones = const.tile([P, 1], FP32R, tag="ones")
            nc.vector.tensor_copy(ones[:], ones32[:])
            cmask = const.tile([P, P], FP32, tag="cmask")
            make_causal_mask(nc, cmask[:], mask_val=-1e10)
            ident32 = const.tile([P, P], FP32, tag="ident32")
            make_identity(nc, ident32[:])
            ident = const.tile([P, P], FP32R, tag="ident")
            nc.vector.tensor_copy(ident[:], ident32[:])
            ln_g1 = const.tile([P, DC], FP32, tag="g1")
            ln_b1 = const.tile([P, DC], FP32, tag="lb1")
            ln_g2 = const.tile([P, DC], FP32, tag="g2")
            ln_b2 = const.tile([P, DC], FP32, tag="lb2")
            bqk_sb = const.tile([P, QKM], FP32, tag="bqk")
            bv_row = const.tile([1, VW], FP32, tag="bvr")
            bv_pl = const.tile([P, VW], FP32, tag="bvp")
            b1_sb = const.tile([P, FC], FP32, tag="b1s")
            b2_sb = const.tile([P, DL // P], FP32, tag="b2s")
            nc.sync.dma_start(ln_g1[:], ln1g[:])
            nc.sync.dma_start(ln_b1[:], ln1b[:])
            nc.sync.dma_start(ln_g2[:], ln2g[:])
            nc.sync.dma_start(ln_b2[:], ln2b[:])
            nc.sync.dma_start(bqk_sb[:], bqk[:])
            nc.sync.dma_start(bv_row[:], bv[:])
            nc.sync.dma_start(b1_sb[:], b1[:])
            nc.sync.dma_start(b2_sb[:], b2[:])
            nc.gpsimd.partition_broadcast(bv_pl[:], bv_row[:])

            # ======== Phase A: LN1 (x^T -> r^T in place, SBUF) ========
            big = tc.alloc_tile_pool(name="big", bufs=1)
            rT = []
            for dc in range(DC):
                t = big.tile([P, Tl], FP32R, tag=f"big{dc}")
                nc.gpsimd.dma_start(t[:], xt[ts(dc, P), :])
                rT.append(t)
            _layernorm_T(nc, tc, rT, ones, ln_g1, ln_b1, cfg)

            # ======== Phase B: qkv projections ========
            with (
                tc.tile_pool(name="ev", bufs=3) as ev,
                tc.tile_pool(name="wvp", bufs=1) as wvp,
                tc.tile_pool(name="wpool", bufs=2) as wpool,
                tc.tile_pool(name="bps", bufs=2, space="PSUM") as bps,
            ):
                for mc in range(QKM):
                    wt = wpool.tile([P, DC, P], FP32R, tag="wqk")
                    nc.gpsimd.dma_start(
                        wt[:],
                        wqkv[:, ts(mc, P)].rearrange("(c k) m -> k c m", k=P),
                    )
                    for tn in range(TN):
                        acc = bps.tile([P, 512], FP32, tag="acc")
                        for dc in range(DC):
                            nc.tensor.matmul(
                                acc[:], wt[:, dc, :], rT[dc][:, ts(tn, 512)],
                                start=(dc == 0), stop=(dc == DC - 1),
                            )
                        st = ev.tile([P, 512], FP32R, tag="qk_ev")
                        nc.vector.tensor_scalar_add(
                            st[:], acc[:], bqk_sb[:, mc : mc + 1]
                        )
                        nc.sync.dma_start(qkT_d[ts(mc, P), ts(tn, 512)], st[:])

                # v natural [T, VW]: lhsT = rT token chunk, rhs = w_v K-chunks
                nvw = (VW + 511) // 512
                for vn in range(nvw):
                    w = min(512, VW - vn * 512)
                    wv = wvp.tile([P, DC, 512], FP32R, tag="wv")
                    nc.gpsimd.dma_start(
                        wv[:, :, :w],
                        wqkv[:, QKM * P + vn * 512 : QKM * P + vn * 512 + w]
                        .rearrange("(c k) m -> k c m", k=P),
                    )
                    for tck in range(TT):
                        acc = bps.tile([P, 512], FP32, tag="acc")
                        for dc in range(DC):
                            nc.tensor.matmul(
                                acc[:, :w], rT[dc][:, ts(tck, P)],
                                wv[:, dc, :w],
                                start=(dc == 0), stop=(dc == DC - 1),
                            )
                        ve = ev.tile([P, 512], FP32R, tag="v_ev")
                        nc.vector.tensor_tensor(
                            ve[:, :w], acc[:, :w],
                            bv_pl[:, vn * 512 : vn * 512 + w], op=OP.add,
                        )
                        nc.sync.dma_start(
                            v_d[ts(tck, P), vn * 512 : vn * 512 + w], ve[:, :w]
                        )
            big.release()  # r^T dead; free 16MB before attention

            # ======== Phase C: causal attention per head ========
            with (
                tc.tile_pool(name="apool", bufs=2) as apool,
                tc.tile_pool(name="vhp", bufs=2) as vhp,
                tc.tile_pool(name="ppool", bufs=2) as ppool,
                tc.tile_pool(name="ptasm", bufs=1) as ptasm,
                tc.tile_pool(name="small", bufs=6) as small,
                tc.tile_pool(name="opool", bufs=1) as opool,
                tc.tile_pool(name="s_ps", bufs=4, space="PSUM") as s_ps,
                tc.tile_pool(name="t_ps", bufs=2, space="PSUM") as t_ps,
                tc.tile_pool(name="o_ps", bufs=2, space="PSUM") as o_ps,
            ):
                oT_sb = [opool.tile([P, Tl], FP32, tag=f"o{h}", name=f"oT{h}") for h in range(HL)]
                for h in range(HL):
                    qT = apool.tile([P, Tl], FP32R, tag="qT")
                    kT = apool.tile([P, Tl], FP32R, tag="kT")
                    nc.sync.dma_start(qT[:], qkT_d[ts(h, P), :])
                    nc.sync.dma_start(kT[:], qkT_d[ts(HL + h, P), :])
                    vh = []
                    for k in range(TT):
                        vt = vhp.tile([P, P], FP32R, tag=f"vh{k}")
                        nc.sync.dma_start(vt[:], v_d[ts(k, P), ts(h, P)])
                        vh.append(vt)
                    for jj in range(TN):
                        pts = [
                            ptasm.tile([P, 512], FP32R, tag=f"pt{k}",
                                       name=f"pt{k}")
                            for k in range(4 * jj + 4)
                        ]
                        for i in range(4 * jj, 4 * jj + 4):
                            nk = i + 1
                            ng = (nk + 3) // 4
                            prow = ppool.tile([P, Tl], FP32R, tag="prow")
                            sg = []
                            for g in range(ng):
                                w = min(512, nk * P - g * 512)
                                st = s_ps.tile([P, 512], FP32, tag="s")
                                nc.tensor.matmul(
                                    st[:, :w], qT[:, ts(i, P)],
                                    kT[:, g * 512 : g * 512 + w],
                                    start=True, stop=True,
                                )
                                sg.append((st, w))
                            dst, dw = sg[-1]
                            nc.vector.tensor_tensor(
                                dst[:, dw - P : dw], dst[:, dw - P : dw],
                                cmask[:], op=OP.add,
                            )
                            mx = small.tile([P, 1], FP32, tag="mx")
                            for g, (st, w) in enumerate(sg):
                                if g == 0:
                                    nc.vector.tensor_reduce(
                                        mx[:], st[:, :w], axis=AX.X, op=OP.max
                                    )
                                else:
                                    m2 = small.tile([P, 1], FP32, tag="mx2")
                                    nc.vector.tensor_reduce(
                                        m2[:], st[:, :w], axis=AX.X, op=OP.max
                                    )
                                    nc.vector.tensor_tensor(
                                        mx[:], mx[:], m2[:], op=OP.max
                                    )
                            nbias = small.tile([P, 1], FP32, tag="nb")
                            nc.vector.tensor_scalar_mul(nbias[:], mx[:], -scale)
                            tot = small.tile([P, 1], FP32, tag="tot")
                            for g, (st, w) in enumerate(sg):
                                acc_o = small.tile([P, 1], FP32, tag="acc_o")
                                nc.scalar.activation(
                                    prow[:, g * 512 : g * 512 + w], st[:, :w],
                                    AF.Exp, bias=nbias[:], scale=scale,
                                    accum_out=acc_o[:],
                                )
                                if g == 0:
                                    nc.vector.tensor_copy(tot[:], acc_o[:])
                                else:
                                    nc.vector.tensor_tensor(
                                        tot[:], tot[:], acc_o[:], op=OP.add
                                    )
                            rcp = small.tile([P, 1], FP32, tag="rcp")
                            nc.vector.reciprocal(rcp[:], tot[:])
                            nc.vector.tensor_scalar_mul(
                                prow[:, : nk * P], prow[:, : nk * P], rcp[:]
                            )
                            col = (i - 4 * jj) * P
                            for k in range(nk):
                                tp = t_ps.tile([P, P], FP32R, tag="tp")
                                nc.tensor.transpose(
                                    tp[:], prow[:, ts(k, P)], ident[:]
                                )
                                nc.vector.tensor_copy(
                                    pts[k][:, col : col + P], tp[:]
                                )
                        op_t = o_ps.tile([P, 512], FP32, tag="o")
                        last = 4 * jj + 3
                        for k in range(4 * jj + 4):
                            c0 = max(0, (k - 4 * jj) * P)
                            nc.tensor.matmul(
                                op_t[:, c0:], vh[k][:], pts[k][:, c0:],
                                start=(k == 0), stop=(k == last),
                            )
                        nc.vector.tensor_copy(oT_sb[h][:, ts(jj, 512)], op_t[:])
                    nc.sync.dma_start(oT_hd[h][:, :], oT_sb[h][:])
                    nc.gpsimd.collective_compute(
                        "AllGather", OP.bypass, replica_groups=GROUPS,
                        ins=[oT_hd[h].opt()], outs=[ag_hd[h].opt()],
                    )

            # ======== Phase D: x2 = x + attn, LN2 -> r2^T (SBUF) ========
            big2 = tc.alloc_tile_pool(name="big2", bufs=1)
            r2 = []
            with tc.tile_pool(name="ldp", bufs=2) as ldp:
                for dc in range(DC):
                    t = big2.tile([P, Tl], FP32R, tag=f"bg{dc}")
                    for tn in range(TN):
                        xa = ldp.tile([P, 512], FP32, tag="xa")
                        xb = ldp.tile([P, 512], FP32, tag="xb")
                        nc.sync.dma_start(xa[:], xt[ts(dc, P), ts(tn, 512)])
                        nc.sync.dma_start(
                            xb[:],
                            ag_hd[dc % HL][ts(dc // HL, P), ts(tn, 512)],
                        )
                        nc.vector.tensor_tensor(
                            t[:, ts(tn, 512)], xa[:], xb[:], op=OP.add
                        )
                    r2.append(t)
            _layernorm_T(nc, tc, r2, ones, ln_g2, ln_b2, cfg)

            # ======== Phase E: h = gelu(r2 @ w1 + b1) -> h_d ========
            with (
                tc.tile_pool(name="eev", bufs=3) as eev,
                tc.tile_pool(name="wpoole", bufs=2) as wpool,
                tc.tile_pool(name="eps", bufs=2, space="PSUM") as eps_p,
            ):
                for fc in range(FC):
                    wt = wpool.tile([P, DC, P], FP32R, tag="wqk")
                    nc.gpsimd.dma_start(
                        wt[:],
                        w1[:, ts(fc, P)].rearrange("(c k) m -> k c m", k=P),
                    )
                    for tn in range(TN):
                        acc = eps_p.tile([P, 512], FP32, tag="acc")
                        for dc in range(DC):
                            nc.tensor.matmul(
                                acc[:], wt[:, dc, :], r2[dc][:, ts(tn, 512)],
                                start=(dc == 0), stop=(dc == DC - 1),
                            )
                        hv = eev.tile([P, 512], FP32R, tag="hev")
                        nc.scalar.activation(
                            hv[:], acc[:], GELU_FUNC,
                            bias=b1_sb[:, fc : fc + 1],
                        )
                        nc.sync.dma_start(h_d[ts(fc, P), ts(tn, 512)], hv[:])
            big2.release()

            # ======== Phase F: partial y^T = h @ w2 -> rs_in ========
            with (
                tc.tile_pool(name="w2c", bufs=1) as w2c,
                tc.tile_pool(name="hsp", bufs=1) as hsp,
                tc.tile_pool(name="fev", bufs=3) as fev,
                tc.tile_pool(name="fps", bufs=2, space="PSUM") as fps,
            ):
                w2t = []
                for fc in range(FC):
                    t = w2c.tile([P, Dd], FP32R, tag=f"w2{fc}")
                    nc.gpsimd.dma_start(t[:], w2[ts(fc, P), :])
                    w2t.append(t)
                for tn in range(TN):
                    hs = [
                        hsp.tile([P, 512], FP32R, tag=f"hs{f_}", name=f"hs{f_}")
                        for f_ in range(FC)
                    ]
                    for fc in range(FC):
                        nc.sync.dma_start(hs[fc][:], h_d[ts(fc, P), ts(tn, 512)])
                    for mc in range(DC):
                        acc = fps.tile([P, 512], FP32, tag="acc")
                        for fc in range(FC):
                            nc.tensor.matmul(
                                acc[:], w2t[fc][:, ts(mc, P)], hs[fc][:],
                                start=(fc == 0), stop=(fc == FC - 1),
                            )
                        yv = fev.tile([P, 512], FP32, tag="yev")
                        nc.vector.tensor_copy(yv[:], acc[:])
                        nc.sync.dma_start(rs_in_td[tn][ts(mc, P), :], yv[:])

            # ======== Final: out = rs + x_res + own attn rows + b2 ========
            with tc.tile_pool(name="fin", bufs=3) as fin:
                for tn in range(TN):
                    nc.gpsimd.collective_compute(
                        "ReduceScatter", OP.add, replica_groups=GROUPS,
                        ins=[rs_in_td[tn].opt()], outs=[rs_out_td[tn].opt()],
                    )
                for tn in range(TN):
                    for q in range(DL // P):
                        a = fin.tile([P, 512], FP32, tag="fa")
                        bt = fin.tile([P, 512], FP32, tag="fb")
                        c = fin.tile([P, 512], FP32, tag="fc")
                        nc.sync.dma_start(a[:], rs_out_td[tn][ts(q, P), :])
                        nc.sync.dma_start(bt[:], xres[ts(q, P), ts(tn, 512)])
                        nc.sync.dma_start(c[:], oT_hd[q][:, ts(tn, 512)])
                        nc.vector.tensor_tensor(a[:], a[:], bt[:], op=OP.add)
                        nc.vector.tensor_tensor(a[:], a[:], c[:], op=OP.add)
                        nc.vector.tensor_scalar_add(
                            a[:], a[:], b2_sb[:, q : q + 1]
                        )
                        nc.sync.dma_start(out[ts(q, P), ts(tn, 512)], a[:])

    nc.compile()
    return nc


_NC_CACHE = {}


def _get_nc(cfg=FULL):
    key = (cfg.T, cfg.D, cfg.FL, cfg.HL)
    if key not in _NC_CACHE:
        _NC_CACHE[key] = build(cfg)
    return _NC_CACHE[key]


def _prep_inputs(x, ln1_scale, ln1_bias, w_qkv, b_qkv, ln2_scale, ln2_bias,
                 w1, b1, w2, b2, cfg=FULL):
    """Host-side sharding: per-core input dicts (core = 4*b + r)."""
    f32 = np.float32
    Dd, FL, HL, DC, FC, QKM, DL, VW = (cfg.D, cfg.FL, cfg.HL, cfg.DC, cfg.FC,
                                       cfg.QKM, cfg.DL, cfg.VW)

    def colmaj(v, nch):
        return np.ascontiguousarray(np.asarray(v, f32).reshape(nch, P).T)

    ln1g = colmaj(ln1_scale, DC)
    ln1b_ = colmaj(ln1_bias, DC)
    ln2g = colmaj(ln2_scale, DC)
    ln2b_ = colmaj(ln2_bias, DC)
    x = np.asarray(x, f32)
    w_qkv = np.asarray(w_qkv, f32)
    b_qkv = np.asarray(b_qkv, f32)
    w1 = np.asarray(w1, f32)
    w2 = np.asarray(w2, f32)
    b1 = np.asarray(b1, f32)
    b2 = np.asarray(b2, f32)

    in_maps = []
    for core in range(NCORES):
        b_, r = divmod(core, TP)
        qs, ks, vs = (r * VW, Dd + r * VW, 2 * Dd + r * VW)
        wqkv_s = np.ascontiguousarray(
            np.concatenate(
                [w_qkv[:, qs : qs + VW], w_qkv[:, ks : ks + VW],
                 w_qkv[:, vs : vs + VW]], axis=1,
            )
        )
        bqk_s = colmaj(
            np.concatenate([b_qkv[qs : qs + VW], b_qkv[ks : ks + VW]]), QKM
        )
        bv_s = np.ascontiguousarray(b_qkv[vs : vs + VW].reshape(1, VW))
        in_maps.append({
            "xt": np.ascontiguousarray(x[b_].T),
            "wqkv": wqkv_s,
            "bqk": bqk_s,
            "bv": bv_s,
            "ln1g": ln1g, "ln1b": ln1b_, "ln2g": ln2g, "ln2b": ln2b_,
            "w1": np.ascontiguousarray(w1[:, r * FL : (r + 1) * FL]),
            "b1": colmaj(b1[r * FL : (r + 1) * FL], FC),
            "w2": np.ascontiguousarray(w2[r * FL : (r + 1) * FL, :]),
            "b2": colmaj(b2[r * DL : (r + 1) * DL], DL // P),
            "xres": np.ascontiguousarray(x[b_][:, r * DL : (r + 1) * DL].T),
        })
    return in_maps


def kernel(**inputs):
    cfg = FULL
    nc = _get_nc(cfg)
    in_maps = _prep_inputs(**inputs, cfg=cfg)
    res = run_bass_kernel_spmd(nc, in_maps, core_ids=list(range(NCORES)))
    y = np.empty((B, cfg.T, cfg.D), np.float32)
    for b_ in range(B):
        yt = np.concatenate(
            [res.results[4 * b_ + r]["out"] for r in range(TP)], axis=0
        )
        y[b_] = yt.T
    return y
